# revision 1
# baseline (speedup 1.0000x reference)
"""Trainium2 Bass kernel for ExodusNet: per-timestep 32->1 dense, ExpLeak scan,
LIF (SingleSpike + MembraneSubtract) over T=100.

Contract: kernel(x, w) takes FULL inputs
    x: (32768, 2, 4, 4, 100) f32, w: (1, 32) f32
returns FULL output (32768, 1, 100) f32 (the spike trains).

Sharding: pure data parallel over the batch dim across 8 NeuronCores
(4096 batches per core), w replicated.

Per-core plan:
  - batch decomposition b = st*512 + k4*128 + p
    (st in [0,8) supertile, k4 in [0,4), p = partition)
  - SBUF x supertile: [128, 12800], col = k4*3200 + f*100 + t.
    Natural batch-major layout: one 2-D DMA, 12.8KB contiguous per
    partition -> HBM line rate.
  - weighted = sum_f w[f] * x[:,f,:] computed jointly:
      * features 0..F_PE-1 on TensorE: stationary diagonals
        Wsel_c = w[c] * I_128 (host-built); fp32 matmuls (4 cyc/col)
        accumulate in PSUM [128, 400].
      * features F_PE..31 on VectorE: tensor_scalar/scalar_tensor_tensor
        MAC with per-partition broadcast w (host-built [128, 32]).
      * combined with one tensor_tensor add per supertile.
  - ExpLeak: tensor_tensor_scan along t (state = alpha*state + i_t),
    then scale by (1-alpha) into u (the reference's exact rounding order
    for the LIF input).
  - LIF: 100 sequential steps on [128, 32] slices covering all 4096
    batches: v = alpha*v + u_t; s = (v >= 1); v = v - s.  Spikes go to a
    t-major staging tile (contiguous writes).
  - Spike tile [128, 3200] dumped contiguously; host transposes.

`reps` repeats the whole pipeline inside one NEFF with an all-engine
barrier in between; wall(reps=R) - wall(reps=1) isolates HW time from
host/compile/transfer overhead for benchmarking.
"""

import numpy as np
from contextlib import ExitStack

import jax
import concourse.bass as bass
import concourse.bacc as bacc
import concourse.mybir as mybir
from concourse import tile

N_CORES = 8
B_FULL = 32768
BS = B_FULL // N_CORES  # 4096 batches per core
T = 100
F = 32
F_PE = 22          # features done on TensorE (fp32 diag matmuls)
ST = 8             # supertiles per core, 512 batches each
K4 = 4             # 128-batch tiles per supertile
COLS = BS // 128 * T  # 3200 staging columns per partition

ALPHA = float(np.exp(-1.0 / 10.0))
ONE_MINUS_ALPHA = float(1.0 - np.exp(-1.0 / 10.0))
THR = 1.0

_DT = mybir.dt.float32


def _build_program(reps: int = 1) -> bass.Bass:
    nc = bacc.Bacc()
    x_in = nc.declare_dram_parameter("x", [BS, 2, 4, 4, T], _DT, isOutput=False)
    # host-precomputed stationary weights: wsel[c] = w[0, c] * I_128
    ws_in = nc.declare_dram_parameter("wsel", [F_PE, 128, 128], _DT, isOutput=False)
    # w broadcast across partitions: wb[p, f] = w[0, f]
    wb_in = nc.declare_dram_parameter("wb", [128, F], _DT, isOutput=False)
    out = nc.declare_dram_parameter("out", [128, COLS], _DT, isOutput=True)

    # x viewed as [st, p, k4, (f t)]
    xs = x_in.rearrange(
        "(st k4 p) c2 h w t -> st p k4 (c2 h w t)", st=ST, k4=K4, p=128
    )

    with ExitStack() as ctx:
        tc = ctx.enter_context(tile.TileContext(nc))
        singles = ctx.enter_context(tc.tile_pool(name="singles", bufs=1))
        xpool = ctx.enter_context(tc.tile_pool(name="xpool", bufs=2))
        upool = ctx.enter_context(tc.tile_pool(name="upool", bufs=3))
        psum = ctx.enter_context(tc.tile_pool(name="psum", bufs=4, space="PSUM"))

        mm = mybir.AluOpType.mult
        ad = mybir.AluOpType.add

        wsel = singles.tile([128, F_PE * 128], _DT)
        wv = wsel.rearrange("p (c m) -> p c m", c=F_PE)
        nc.sync.dma_start(out=wv, in_=ws_in.rearrange("c p m -> p c m"))
        wb = singles.tile([128, F], _DT)
        nc.sync.dma_start(out=wb, in_=wb_in[:, :])

        alphas = singles.tile([128, T], _DT)
        nc.vector.memset(alphas, ALPHA)

        u_t = singles.tile([128, COLS], _DT)   # (1-alpha) * syn
        s_t = singles.tile([128, COLS], _DT)   # v_pre then spikes, t-major
        ym_t = singles.tile([128, 32], _DT)    # s - v = negated post-reset state

        for rep in range(reps):
            if rep > 0:
                tc.strict_bb_all_engine_barrier()
            nc.vector.memset(ym_t, 0.0)

            for st in range(ST):
                xsup = xpool.tile([128, K4 * F * T], _DT)
                xsv = xsup.rearrange("p (k4 ct) -> p k4 ct", k4=K4)
                nc.sync.dma_start(out=xsv, in_=xs[st])

                # TensorE: features 0..F_PE-1 accumulate into PSUM
                pt = psum.tile([128, K4 * T], _DT)
                for c in range(F_PE):
                    nc.tensor.matmul(
                        pt,
                        wv[:, c, :],
                        xsv[:, :, T * c : T * (c + 1)],
                        start=(c == 0),
                        stop=(c == F_PE - 1),
                        tile_position=(0, 0),
                    )

                # VectorE: features F_PE..31 accumulate into upart
                upart = upool.tile([128, K4 * T], _DT)
                nc.vector.tensor_scalar(
                    upart,
                    xsv[:, :, T * F_PE : T * (F_PE + 1)],
                    wb[:, F_PE : F_PE + 1],
                    None,
                    mm,
                )
                for c in range(F_PE + 1, F):
                    nc.vector.scalar_tensor_tensor(
                        out=upart,
                        in0=xsv[:, :, T * c : T * (c + 1)],
                        scalar=wb[:, c : c + 1],
                        in1=upart,
                        op0=mm,
                        op1=ad,
                    )
                # combine PE + DVE partials
                nc.vector.tensor_tensor(upart, upart, pt, ad)

                base = st * K4 * T
                for k4 in range(K4):
                    nc.vector.tensor_tensor_scan(
                        out=u_t[:, base + T * k4 : base + T * (k4 + 1)],
                        data0=alphas,
                        data1=upart[:, T * k4 : T * (k4 + 1)],
                        initial=0.0,
                        op0=mm,
                        op1=ad,
                    )
            # LIF, 2 ops/step: V_t = (-alpha)*Ym + u_t  (Ym = s-v post-reset,
            # negated); Ym = (V_t >= 1) - V_t.  V holds pre-reset potentials;
            # spikes recovered in bulk afterwards: s = (V >= 1).
            uv = u_t.rearrange("p (k t) -> p k t", t=T)
            sv = s_t.rearrange("p (t k) -> p t k", t=T)
            for t in range(T):
                nc.vector.scalar_tensor_tensor(
                    out=sv[:, t, :],
                    in0=ym_t,
                    scalar=-ALPHA,
                    in1=uv[:, :, t],
                    op0=mm,
                    op1=ad,
                )
                nc.vector.scalar_tensor_tensor(
                    out=ym_t,
                    in0=sv[:, t, :],
                    scalar=THR,
                    in1=sv[:, t, :],
                    op0=mybir.AluOpType.is_ge,
                    op1=mybir.AluOpType.subtract,
                )
            for j in range(8):
                nc.vector.tensor_scalar(
                    s_t[:, 400 * j : 400 * (j + 1)],
                    s_t[:, 400 * j : 400 * (j + 1)],
                    THR,
                    None,
                    mybir.AluOpType.is_ge,
                )

            nc.sync.dma_start(out=out[:, :], in_=s_t)

    nc.finalize()
    return nc


class _Launcher:
    """Compiled SPMD launcher (mirrors bass2jax.run_bass_via_pjrt but keeps
    the jitted executable so repeat calls don't recompile)."""

    def __init__(self, nc: bass.Bass, donate: bool = True):
        from jax.experimental.shard_map import shard_map
        from jax.sharding import Mesh, PartitionSpec
        from concourse.bass2jax import (
            _bass_exec_p,
            install_neuronx_cc_hook,
            partition_id_tensor,
        )

        install_neuronx_cc_hook()
        self.nc = nc
        partition_name = (
            nc.partition_id_tensor.name if nc.partition_id_tensor else None
        )
        in_names: list[str] = []
        out_names: list[str] = []
        out_avals: list[jax.core.ShapedArray] = []
        zero_shapes: list[tuple] = []
        for alloc in nc.m.functions[0].allocations:
            if not isinstance(alloc, mybir.MemoryLocationSet):
                continue
            name = alloc.memorylocations[0].name
            if alloc.kind == "ExternalInput":
                if name != partition_name:
                    in_names.append(name)
            elif alloc.kind == "ExternalOutput":
                out_names.append(name)
                shape = tuple(alloc.tensor_shape)
                dtype = mybir.dt.np(alloc.dtype)
                out_avals.append(jax.core.ShapedArray(shape, dtype))
                zero_shapes.append((shape, dtype))
        self.in_names = list(in_names)
        self.out_names = out_names
        self.out_avals = out_avals
        self.zero_shapes = zero_shapes
        n_params = len(in_names)
        all_in_names = list(in_names) + list(out_names)
        if partition_name is not None:
            all_in_names.append(partition_name)

        def _body(*args):
            operands = list(args)
            if partition_name is not None:
                operands.append(partition_id_tensor())
            outs = _bass_exec_p.bind(
                *operands,
                out_avals=tuple(out_avals),
                in_names=tuple(all_in_names),
                out_names=tuple(out_names),
                lowering_input_output_aliases=(),
                sim_require_finite=True,
                sim_require_nnan=True,
                nc=nc,
            )
            return tuple(outs)

        devices = jax.devices()[:N_CORES]
        self.mesh = Mesh(np.asarray(devices), ("core",))
        n_outs = len(out_names)
        donate_argnums = (
            tuple(range(n_params, n_params + n_outs)) if donate else ()
        )
        in_specs = (PartitionSpec("core"),) * (n_params + n_outs)
        out_specs = (PartitionSpec("core"),) * n_outs
        self.sharded = jax.jit(
            shard_map(
                _body,
                mesh=self.mesh,
                in_specs=in_specs,
                out_specs=out_specs,
                check_rep=False,
            ),
            donate_argnums=donate_argnums,
            keep_unused=True,
        )

    def zeros(self):
        return [
            np.zeros((N_CORES * s[0], *s[1:]), d) for (s, d) in self.zero_shapes
        ]

    def __call__(self, concat_inputs):
        out_arrs = self.sharded(*concat_inputs, *self.zeros())
        return [np.asarray(o) for o in out_arrs]


_launchers: dict[tuple, _Launcher] = {}


def _get_launcher(reps: int = 1, donate: bool = True) -> _Launcher:
    key = (reps, donate)
    if key not in _launchers:
        _launchers[key] = _Launcher(_build_program(reps), donate=donate)
    return _launchers[key]


def _make_wsel(w: np.ndarray) -> np.ndarray:
    ws = np.zeros((F_PE, 128, 128), dtype=np.float32)
    idx = np.arange(128)
    for c in range(F_PE):
        ws[c, idx, idx] = w[0, c]
    return ws


def _unscramble(full_out: np.ndarray) -> np.ndarray:
    # full_out: [8*128, 3200]; per-core [p, t*32+k] = spike[k*128 + p, t]
    return (
        full_out.reshape(N_CORES, 128, T, BS // 128)
        .transpose(0, 3, 1, 2)
        .reshape(B_FULL, 1, T)
    )


def _prep_inputs(x, w):
    x = np.ascontiguousarray(np.asarray(x, dtype=np.float32))
    w = np.ascontiguousarray(np.asarray(w, dtype=np.float32))
    assert x.shape == (B_FULL, 2, 4, 4, T), x.shape
    assert w.shape == (1, F), w.shape
    wsc = (np.float32(ONE_MINUS_ALPHA) * w).astype(np.float32)
    ws = _make_wsel(wsc)
    ws_rep = np.broadcast_to(ws, (N_CORES, *ws.shape)).reshape(
        N_CORES * F_PE, 128, 128
    )
    wb = np.broadcast_to(wsc[0], (128, F))
    wb_rep = np.broadcast_to(wb, (N_CORES, 128, F)).reshape(N_CORES * 128, F)
    return [
        x,
        np.ascontiguousarray(ws_rep),
        np.ascontiguousarray(wb_rep),
    ]


def run(x, w, reps: int = 1):
    launcher = _get_launcher(reps)
    concat_in = _prep_inputs(x, w)
    # input order must match the BIR ExternalInput declaration order
    assert launcher.in_names == ["x", "wsel", "wb"], launcher.in_names
    outs = launcher(concat_in)
    return _unscramble(outs[0])


def kernel(x, w):
    return run(x, w, reps=1)



# revision 4
# speedup vs baseline: 1.2030x; 1.2030x over previous
"""Trainium2 Bass kernel for ExodusNet: per-timestep 32->1 dense, ExpLeak scan,
LIF (SingleSpike + MembraneSubtract) over T=100.

Contract: kernel(x, w) takes FULL inputs
    x: (32768, 2, 4, 4, 100) f32, w: (1, 32) f32
returns FULL output (32768, 1, 100) f32 (the spike trains).

Sharding: pure data parallel over the batch dim across 8 NeuronCores
(4096 batches per core), w replicated.

Per-core plan (v2, pipelined halves):
  - batch decomposition b = st*512 + k4*128 + p  (st supertile, k4 in
    [0,4), p = partition).  k-block index k = st*4 + k4 in [0,32);
    halves h = k // 16, kh = k % 16.
  - x supertile [128, 12800] via one 6.55MB DMA (qSP); the LAST supertile
    is DMA'd in 4 k4-granular chunks so tail compute starts early.
  - weighted = sum_f w[f] * x[:,f,:]: features 0..F_PE-1 on TensorE
    (stationary diagonal fp32 matmuls accumulating in PSUM), features
    F_PE..31 on VectorE (tensor_scalar + scalar_tensor_tensor MACs),
    combined with one tensor_tensor add.  w is pre-scaled by (1-alpha)
    on host so the ExpLeak scan directly yields the LIF drive u.
  - ExpLeak: tensor_tensor_scan along t per k-block, writing u into a
    t-major staging tile u2 [128, 3200] (col = h*1600 + t*16 + kh).
  - LIF per half h (16 k-blocks, all 2048 batches of the half per step):
    v_t stored t-major in v2 (same layout).  One DVE op per step:
      custom DVE op LIF_STEP_ANT: v_t = (v_{t-1} - (v_{t-1} >= 1))*alpha + u_t
    (fallback: two scalar_tensor_tensor ops per step via the negated
    post-reset state ym = s - v).  Half 0's chain is emitted between
    supertile 3 and 4 so it overlaps the second half of the DMA stream;
    only half 1's chain is in the tail.
  - Spikes: in-place is_ge over v2 in t-chunks, each followed by a
    contiguous output DMA on the second HWDGE queue (qAct) so writeback
    overlaps the remaining chain.

`reps` repeats the pipeline inside one NEFF with an all-engine barrier
between reps; wall(reps=R) - wall(reps=1) isolates HW time.
"""

import numpy as np
from contextlib import ExitStack

import jax
import concourse.bass as bass
import concourse.bacc as bacc
import concourse.mybir as mybir
from concourse import tile

# ---------------------------------------------------------------------------
# Custom DVE op: one fused LIF step per instruction.
#   out = (Src0 - (Src0 >= 1)) * C0 + Src1
# Matches the reference's rounding order exactly: w = v - s (exact via
# Sterbenz for v in [1,2)), m = round(alpha*w), v' = round(m + u).
# Registered at import into the process-local dve_ops registry (the
# documented extension point; the uop table rides the NEFF).
# ---------------------------------------------------------------------------
from concourse import dve_ops as _dve_ops
from concourse.dve_spec import Spec as _Spec, Src0 as _S0, Src1 as _S1, C0 as _C0, C1 as _C1, lower as _lower
from concourse.dve_uop import DveOpSpec as _DveOpSpec


def _register_lif_step():
    name = "LIF_STEP_ANT"
    if name in _dve_ops._SUB_OPCODE_FOR_NAME:
        return next(op for op in _dve_ops.OPS if op.name == name)
    spec = _Spec(
        body=(_S0 - (_S0 >= _C1)) * _C0 + _S1,
        reference=lambda in0, in1, s0, s1, imm2: (
            (in0 - (in0 >= np.float32(s1)).astype(np.float32)) * np.float32(s0)
            + in1
        ).astype(np.float32),
    )
    row = _dve_ops._CUSTOM_DVE_ROW_BASE + len(_dve_ops.OPS)
    assert row < 0x20
    _dve_ops._SUB_OPCODE_FOR_NAME[name] = row
    shas = {
        ver: _DveOpSpec(
            name=name, opcode=row, uops=_lower(spec, ver=ver), rd1_en=True
        ).sha(ver)
        for ver in ("v3", "v4")
    }
    op = _dve_ops.DveOp(name, spec, subdim=False, uops_sha=shas)
    _dve_ops.OPS.append(op)
    _dve_ops.CUSTOM_DVE_SPECS[name] = spec
    return op


LIF_STEP = _register_lif_step()

USE_CUSTOM_LIF = True  # False -> 2x scalar_tensor_tensor per step fallback

N_CORES = 8
B_FULL = 32768
BS = B_FULL // N_CORES  # 4096 batches per core
T = 100
F = 32
F_PE = 22          # features done on TensorE (fp32 diag matmuls)
ST = 8             # supertiles per core, 512 batches each
K4 = 4             # 128-batch tiles per supertile
NKB = ST * K4      # 32 k-blocks
HALF_KB = NKB // 2  # 16 k-blocks per half
COLS = NKB * T     # 3200 staging columns per partition

ALPHA = float(np.exp(-1.0 / 10.0))
ONE_MINUS_ALPHA = float(1.0 - np.exp(-1.0 / 10.0))
THR = 1.0

_DT = mybir.dt.float32


def _build_program(reps: int = 1) -> bass.Bass:
    nc = bacc.Bacc()
    x_in = nc.declare_dram_parameter("x", [BS, 2, 4, 4, T], _DT, isOutput=False)
    ws_in = nc.declare_dram_parameter("wsel", [F_PE, 128, 128], _DT, isOutput=False)
    wb_in = nc.declare_dram_parameter("wb", [128, F], _DT, isOutput=False)
    out = nc.declare_dram_parameter("out", [128, COLS], _DT, isOutput=True)

    # x viewed as [st, p, k4, (f t)]
    xs = x_in.rearrange(
        "(st k4 p) c2 h w t -> st p k4 (c2 h w t)", st=ST, k4=K4, p=128
    )

    mm = mybir.AluOpType.mult
    ad = mybir.AluOpType.add

    with ExitStack() as ctx:
        tc = ctx.enter_context(tile.TileContext(nc))
        singles = ctx.enter_context(tc.tile_pool(name="singles", bufs=1))
        xpool = ctx.enter_context(tc.tile_pool(name="xpool", bufs=2))
        upool = ctx.enter_context(tc.tile_pool(name="upool", bufs=3))
        psum = ctx.enter_context(tc.tile_pool(name="psum", bufs=4, space="PSUM"))

        wsel = singles.tile([128, F_PE * 128], _DT)
        wv = wsel.rearrange("p (c m) -> p c m", c=F_PE)
        nc.sync.dma_start(out=wv, in_=ws_in.rearrange("c p m -> p c m"))
        wb = singles.tile([128, F], _DT)
        nc.sync.dma_start(out=wb, in_=wb_in[:, :])

        alphas = singles.tile([128, T], _DT)
        nc.vector.memset(alphas, ALPHA)

        u2 = singles.tile([128, COLS], _DT)   # LIF drive, col = h*1600+t*16+kh
        v2 = singles.tile([128, COLS], _DT)   # pre-reset potentials -> spikes
        u2v = u2.rearrange("p (h t kh) -> p h t kh", h=2, t=T)
        v2v = v2.rearrange("p (h t kh) -> p h t kh", h=2, t=T)
        if not USE_CUSTOM_LIF:
            ym = singles.tile([128, HALF_KB], _DT)

        def weighted_and_scan(st, xsv, k4s, fd):
            """PE + DVE weighted sum and ExpLeak scans for k4 groups `k4s` of
            supertile `st`; fd = T*len(k4s) free-dim per op."""
            pt = psum.tile([128, K4 * T], _DT)
            upart = upool.tile([128, K4 * T], _DT)
            c0, c1 = k4s[0], k4s[-1] + 1
            psl = slice(T * c0, T * c1)
            for c in range(F_PE):
                nc.tensor.matmul(
                    pt[:, psl],
                    wv[:, c, :],
                    xsv[:, c0:c1, T * c : T * (c + 1)],
                    start=(c == 0),
                    stop=(c == F_PE - 1),
                    tile_position=(0, 0),
                )
            nc.vector.tensor_scalar(
                upart[:, psl],
                xsv[:, c0:c1, T * F_PE : T * (F_PE + 1)],
                wb[:, F_PE : F_PE + 1],
                None,
                mm,
            )
            for c in range(F_PE + 1, F):
                nc.vector.scalar_tensor_tensor(
                    out=upart[:, psl],
                    in0=xsv[:, c0:c1, T * c : T * (c + 1)],
                    scalar=wb[:, c : c + 1],
                    in1=upart[:, psl],
                    op0=mm,
                    op1=ad,
                )
            nc.vector.tensor_tensor(upart[:, psl], upart[:, psl], pt[:, psl], ad)
            for k4 in k4s:
                k = st * K4 + k4
                h, kh = k // HALF_KB, k % HALF_KB
                nc.vector.tensor_tensor_scan(
                    out=u2v[:, h, :, kh],
                    data0=alphas,
                    data1=upart[:, T * k4 : T * (k4 + 1)],
                    initial=0.0,
                    op0=mm,
                    op1=ad,
                )

        def lif_half(h):
            """Sequential LIF chain for half h; spikes + writeback in 2
            t-chunks on the second DMA queue so they overlap the chain."""
            base = h * HALF_KB * T

            def flush(t_lo, t_hi):
                a, b = base + t_lo * HALF_KB, base + t_hi * HALF_KB
                nc.vector.tensor_scalar(
                    v2[:, a:b], v2[:, a:b], THR, None, mybir.AluOpType.is_ge
                )
                nc.scalar.dma_start(out=out[:, a:b], in_=v2[:, a:b])

            # NB: the mid-chain flush is emitted only after the op for
            # t = T//2 so the in-place is_ge never clobbers a potential
            # (v_{T//2-1}) that the next chain step still reads.
            if USE_CUSTOM_LIF:
                nc.vector.tensor_copy(v2v[:, h, 0, :], u2v[:, h, 0, :])
                for t in range(1, T):
                    nc.vector._custom_dve(
                        LIF_STEP,
                        out=v2v[:, h, t, :],
                        in0=v2v[:, h, t - 1, :],
                        in1=u2v[:, h, t, :],
                        s0=ALPHA,
                        s1=THR,
                    )
                    if t == T // 2:
                        flush(0, T // 2)
                flush(T // 2, T)
            else:
                nc.vector.memset(ym, 0.0)
                for t in range(T):
                    nc.vector.scalar_tensor_tensor(
                        out=v2v[:, h, t, :],
                        in0=ym,
                        scalar=-ALPHA,
                        in1=u2v[:, h, t, :],
                        op0=mm,
                        op1=ad,
                    )
                    nc.vector.scalar_tensor_tensor(
                        out=ym,
                        in0=v2v[:, h, t, :],
                        scalar=THR,
                        in1=v2v[:, h, t, :],
                        op0=mybir.AluOpType.is_ge,
                        op1=mybir.AluOpType.subtract,
                    )
                    if t == T // 2:
                        flush(0, T // 2)
                flush(T // 2, T)

        for rep in range(reps):
            if rep > 0:
                tc.strict_bb_all_engine_barrier()
            for st in range(ST):
                xsup = xpool.tile([128, K4 * F * T], _DT)
                xsv = xsup.rearrange("p (k4 ct) -> p k4 ct", k4=K4)
                if st < ST - 1:
                    nc.sync.dma_start(out=xsv, in_=xs[st])
                    weighted_and_scan(st, xsv, [0, 1, 2, 3], K4 * T)
                else:
                    # last supertile: k4-granular DMA + compute for a short tail
                    for k4 in range(K4):
                        nc.sync.dma_start(
                            out=xsv[:, k4, :], in_=xs[st][:, k4, :]
                        )
                        weighted_and_scan(st, xsv, [k4], T)
                if st == ST // 2 - 1:
                    lif_half(0)
            lif_half(1)

    nc.finalize()
    return nc


class _Launcher:
    """Compiled SPMD launcher (mirrors bass2jax.run_bass_via_pjrt but keeps
    the jitted executable so repeat calls don't recompile)."""

    def __init__(self, nc: bass.Bass, donate: bool = True):
        from jax.experimental.shard_map import shard_map
        from jax.sharding import Mesh, PartitionSpec
        from concourse.bass2jax import (
            _bass_exec_p,
            install_neuronx_cc_hook,
            partition_id_tensor,
        )

        install_neuronx_cc_hook()
        self.nc = nc
        partition_name = (
            nc.partition_id_tensor.name if nc.partition_id_tensor else None
        )
        in_names: list[str] = []
        out_names: list[str] = []
        out_avals: list[jax.core.ShapedArray] = []
        zero_shapes: list[tuple] = []
        for alloc in nc.m.functions[0].allocations:
            if not isinstance(alloc, mybir.MemoryLocationSet):
                continue
            name = alloc.memorylocations[0].name
            if alloc.kind == "ExternalInput":
                if name != partition_name:
                    in_names.append(name)
            elif alloc.kind == "ExternalOutput":
                out_names.append(name)
                shape = tuple(alloc.tensor_shape)
                dtype = mybir.dt.np(alloc.dtype)
                out_avals.append(jax.core.ShapedArray(shape, dtype))
                zero_shapes.append((shape, dtype))
        self.in_names = list(in_names)
        self.out_names = out_names
        self.out_avals = out_avals
        self.zero_shapes = zero_shapes
        n_params = len(in_names)
        all_in_names = list(in_names) + list(out_names)
        if partition_name is not None:
            all_in_names.append(partition_name)

        def _body(*args):
            operands = list(args)
            if partition_name is not None:
                operands.append(partition_id_tensor())
            outs = _bass_exec_p.bind(
                *operands,
                out_avals=tuple(out_avals),
                in_names=tuple(all_in_names),
                out_names=tuple(out_names),
                lowering_input_output_aliases=(),
                sim_require_finite=True,
                sim_require_nnan=True,
                nc=nc,
            )
            return tuple(outs)

        devices = jax.devices()[:N_CORES]
        self.mesh = Mesh(np.asarray(devices), ("core",))
        n_outs = len(out_names)
        donate_argnums = (
            tuple(range(n_params, n_params + n_outs)) if donate else ()
        )
        in_specs = (PartitionSpec("core"),) * (n_params + n_outs)
        out_specs = (PartitionSpec("core"),) * n_outs
        self.sharded = jax.jit(
            shard_map(
                _body,
                mesh=self.mesh,
                in_specs=in_specs,
                out_specs=out_specs,
                check_rep=False,
            ),
            donate_argnums=donate_argnums,
            keep_unused=True,
        )

    def zeros(self):
        return [
            np.zeros((N_CORES * s[0], *s[1:]), d) for (s, d) in self.zero_shapes
        ]

    def __call__(self, concat_inputs):
        out_arrs = self.sharded(*concat_inputs, *self.zeros())
        return [np.asarray(o) for o in out_arrs]


_launchers: dict[tuple, _Launcher] = {}


def _get_launcher(reps: int = 1, donate: bool = True) -> _Launcher:
    key = (reps, donate)
    if key not in _launchers:
        _launchers[key] = _Launcher(_build_program(reps), donate=donate)
    return _launchers[key]


def _make_wsel(w: np.ndarray) -> np.ndarray:
    ws = np.zeros((F_PE, 128, 128), dtype=np.float32)
    idx = np.arange(128)
    for c in range(F_PE):
        ws[c, idx, idx] = w[0, c]
    return ws


def _unscramble(full_out: np.ndarray) -> np.ndarray:
    # per-core col = h*1600 + t*16 + kh;  batch-within-core = (h*16+kh)*128 + p
    return (
        full_out.reshape(N_CORES, 128, 2, T, HALF_KB)
        .transpose(0, 2, 4, 1, 3)
        .reshape(B_FULL, 1, T)
    )


def _prep_inputs(x, w):
    x = np.ascontiguousarray(np.asarray(x, dtype=np.float32))
    w = np.ascontiguousarray(np.asarray(w, dtype=np.float32))
    assert x.shape == (B_FULL, 2, 4, 4, T), x.shape
    assert w.shape == (1, F), w.shape
    wsc = (np.float32(ONE_MINUS_ALPHA) * w).astype(np.float32)
    ws = _make_wsel(wsc)
    ws_rep = np.broadcast_to(ws, (N_CORES, *ws.shape)).reshape(
        N_CORES * F_PE, 128, 128
    )
    wb = np.broadcast_to(wsc[0], (128, F))
    wb_rep = np.broadcast_to(wb, (N_CORES, 128, F)).reshape(N_CORES * 128, F)
    return [
        x,
        np.ascontiguousarray(ws_rep),
        np.ascontiguousarray(wb_rep),
    ]


def run(x, w, reps: int = 1):
    launcher = _get_launcher(reps)
    concat_in = _prep_inputs(x, w)
    # input order must match the BIR ExternalInput declaration order
    assert launcher.in_names == ["x", "wsel", "wb"], launcher.in_names
    outs = launcher(concat_in)
    return _unscramble(outs[0])


def kernel(x, w):
    return run(x, w, reps=1)


# revision 8
# speedup vs baseline: 1.3154x; 1.0934x over previous
"""Trainium2 Bass kernel for ExodusNet: per-timestep 32->1 dense, ExpLeak scan,
LIF (SingleSpike + MembraneSubtract) over T=100.

Contract: kernel(x, w) takes FULL inputs
    x: (32768, 2, 4, 4, 100) f32, w: (1, 32) f32
returns FULL output (32768, 1, 100) f32 (the spike trains).

Sharding: pure data parallel over the batch dim across 8 NeuronCores
(4096 batches per core), w replicated.

Per-core plan (v2, pipelined halves):
  - batch decomposition b = st*512 + k4*128 + p  (st supertile, k4 in
    [0,4), p = partition).  k-block index k = st*4 + k4 in [0,32);
    halves h = k // 16, kh = k % 16.
  - x supertile [128, 12800] via one 6.55MB DMA (qSP); the LAST supertile
    is DMA'd in 4 k4-granular chunks so tail compute starts early.
  - weighted = sum_f w[f] * x[:,f,:]: features 0..F_PE-1 on TensorE
    (stationary diagonal fp32 matmuls accumulating in PSUM), features
    F_PE..31 on VectorE (tensor_scalar + scalar_tensor_tensor MACs),
    combined with one tensor_tensor add.  w is pre-scaled by (1-alpha)
    on host so the ExpLeak scan directly yields the LIF drive u.
  - ExpLeak: tensor_tensor_scan along t per k-block, writing u into a
    t-major staging tile u2 [128, 3200] (col = h*1600 + t*16 + kh).
  - LIF per half h (16 k-blocks, all 2048 batches of the half per step):
    v_t stored t-major in v2 (same layout).  One DVE op per step:
      custom DVE op LIF_STEP_ANT: v_t = (v_{t-1} - (v_{t-1} >= 1))*alpha + u_t
    (fallback: two scalar_tensor_tensor ops per step via the negated
    post-reset state ym = s - v).  Half 0's chain is emitted between
    supertile 3 and 4 so it overlaps the second half of the DMA stream;
    only half 1's chain is in the tail.
  - Spikes: in-place is_ge over v2 in t-chunks, each followed by a
    contiguous output DMA on the second HWDGE queue (qAct) so writeback
    overlaps the remaining chain.

`reps` repeats the pipeline inside one NEFF with an all-engine barrier
between reps; wall(reps=R) - wall(reps=1) isolates HW time.
"""

import numpy as np
from contextlib import ExitStack

import jax
import concourse.bass as bass
import concourse.bacc as bacc
import concourse.mybir as mybir
from concourse import tile

# ---------------------------------------------------------------------------
# Custom DVE op: one fused LIF step per instruction.
#   out = (Src0 - (Src0 >= 1)) * C0 + Src1
# Matches the reference's rounding order exactly: w = v - s (exact via
# Sterbenz for v in [1,2)), m = round(alpha*w), v' = round(m + u).
# Registered at import into the process-local dve_ops registry (the
# documented extension point; the uop table rides the NEFF).
# ---------------------------------------------------------------------------
from concourse import dve_ops as _dve_ops
from concourse.dve_spec import Spec as _Spec, Src0 as _S0, Src1 as _S1, C0 as _C0, C1 as _C1, lower as _lower
from concourse.dve_uop import DveOpSpec as _DveOpSpec


def _register_lif_step():
    name = "LIF_STEP_ANT"
    if name in _dve_ops._SUB_OPCODE_FOR_NAME:
        return next(op for op in _dve_ops.OPS if op.name == name)
    spec = _Spec(
        body=(_S0 - (_S0 >= _C1)) * _C0 + _S1,
        reference=lambda in0, in1, s0, s1, imm2: (
            (in0 - (in0 >= np.float32(s1)).astype(np.float32)) * np.float32(s0)
            + in1
        ).astype(np.float32),
    )
    row = _dve_ops._CUSTOM_DVE_ROW_BASE + len(_dve_ops.OPS)
    assert row < 0x20
    _dve_ops._SUB_OPCODE_FOR_NAME[name] = row
    shas = {
        ver: _DveOpSpec(
            name=name, opcode=row, uops=_lower(spec, ver=ver), rd1_en=True
        ).sha(ver)
        for ver in ("v3", "v4")
    }
    op = _dve_ops.DveOp(name, spec, subdim=False, uops_sha=shas)
    _dve_ops.OPS.append(op)
    _dve_ops.CUSTOM_DVE_SPECS[name] = spec
    return op


LIF_STEP = _register_lif_step()

USE_CUSTOM_LIF = True  # False -> 2x scalar_tensor_tensor per step fallback

N_CORES = 8
B_FULL = 32768
BS = B_FULL // N_CORES  # 4096 batches per core
T = 100
F = 32
F_PE = 22          # features done on TensorE (fp32 diag matmuls)
ST = 8             # supertiles per core, 512 batches each
K4 = 4             # 128-batch tiles per supertile
NKB = ST * K4      # 32 k-blocks
HALF_KB = NKB // 2  # 16 k-blocks per half
COLS = NKB * T     # 3200 staging columns per partition

ALPHA = float(np.exp(-1.0 / 10.0))
ONE_MINUS_ALPHA = float(1.0 - np.exp(-1.0 / 10.0))
THR = 1.0

_DT = mybir.dt.float32


def _build_program(reps: int = 1) -> bass.Bass:
    nc = bacc.Bacc()
    x_in = nc.declare_dram_parameter("x", [BS, 2, 4, 4, T], _DT, isOutput=False)
    ws_in = nc.declare_dram_parameter("wsel", [F_PE, 128, 128], _DT, isOutput=False)
    wb_in = nc.declare_dram_parameter("wb", [128, F], _DT, isOutput=False)
    out = nc.declare_dram_parameter("out", [128, COLS], _DT, isOutput=True)

    # x viewed as [st, p, k4, (f t)]
    xs = x_in.rearrange(
        "(st k4 p) c2 h w t -> st p k4 (c2 h w t)", st=ST, k4=K4, p=128
    )

    mm = mybir.AluOpType.mult
    ad = mybir.AluOpType.add

    with ExitStack() as ctx:
        tc = ctx.enter_context(tile.TileContext(nc))
        singles = ctx.enter_context(tc.tile_pool(name="singles", bufs=1))
        xpool = ctx.enter_context(tc.tile_pool(name="xpool", bufs=2))
        upool = ctx.enter_context(tc.tile_pool(name="upool", bufs=3))
        psum = ctx.enter_context(tc.tile_pool(name="psum", bufs=4, space="PSUM"))

        wsel = singles.tile([128, F_PE * 128], _DT)
        wv = wsel.rearrange("p (c m) -> p c m", c=F_PE)
        nc.sync.dma_start(out=wv, in_=ws_in.rearrange("c p m -> p c m"))
        wb = singles.tile([128, F], _DT)
        nc.sync.dma_start(out=wb, in_=wb_in[:, :])

        alphas = singles.tile([128, T], _DT)
        nc.vector.memset(alphas, ALPHA)

        u2 = singles.tile([128, COLS], _DT)   # LIF drive, col = h*1600+t*16+kh
        v2 = singles.tile([128, COLS], _DT)   # pre-reset potentials -> spikes
        u2v = u2.rearrange("p (h t kh) -> p h t kh", h=2, t=T)
        v2v = v2.rearrange("p (h t kh) -> p h t kh", h=2, t=T)
        if not USE_CUSTOM_LIF:
            ym = singles.tile([128, HALF_KB], _DT)

        def weighted_and_scan(st, xsv, k4s):
            """PE + DVE weighted sum and ExpLeak scans for k4 groups `k4s` of
            supertile `st`.  DVE feature MACs use the production
            AFFINE_THEN_ADD custom op: (x_c*w_c + 0) + partial — same
            rounding sequence as scalar_tensor_tensor (the +0 is exact) at
            lower per-op overhead."""
            pt = psum.tile([128, K4 * T], _DT)
            upart = upool.tile([128, K4 * T], _DT)
            c0, c1 = k4s[0], k4s[-1] + 1
            psl = slice(T * c0, T * c1)
            for c in range(F_PE):
                nc.tensor.matmul(
                    pt[:, psl],
                    wv[:, c, :],
                    xsv[:, c0:c1, T * c : T * (c + 1)],
                    start=(c == 0),
                    stop=(c == F_PE - 1),
                    tile_position=(0, 0),
                )
            nc.vector.tensor_scalar(
                upart[:, psl],
                xsv[:, c0:c1, T * F_PE : T * (F_PE + 1)],
                wb[:, F_PE : F_PE + 1],
                None,
                mm,
            )
            for c in range(F_PE + 1, F):
                nc.vector.affine_then_add(
                    out=upart[:, psl],
                    in0=xsv[:, c0:c1, T * c : T * (c + 1)],
                    in1=upart[:, psl],
                    scale=wb[:, c : c + 1],
                    bias=0.0,
                )
            nc.vector.tensor_tensor(upart[:, psl], upart[:, psl], pt[:, psl], ad)
            for k4 in k4s:
                k = st * K4 + k4
                h, kh = k // HALF_KB, k % HALF_KB
                nc.vector.tensor_tensor_scan(
                    out=u2v[:, h, :, kh],
                    data0=alphas,
                    data1=upart[:, T * k4 : T * (k4 + 1)],
                    initial=0.0,
                    op0=mm,
                    op1=ad,
                )

        def lif_half(h):
            """Sequential LIF chain for half h; spikes + writeback in 2
            t-chunks on the second DMA queue so they overlap the chain."""
            base = h * HALF_KB * T

            def flush(t_lo, t_hi):
                a, b = base + t_lo * HALF_KB, base + t_hi * HALF_KB
                nc.vector.tensor_scalar(
                    v2[:, a:b], v2[:, a:b], THR, None, mybir.AluOpType.is_ge
                )
                nc.scalar.dma_start(out=out[:, a:b], in_=v2[:, a:b])

            # flush boundaries: a flush of [a, b) is only emitted after the
            # chain op for t == b (which consumes v_{b-1}) so the in-place
            # is_ge never clobbers a still-needed potential.  The last chunk
            # is small to shorten the post-chain tail.
            t_flush = {45: (0, 45), 90: (45, 90)}

            if USE_CUSTOM_LIF:
                nc.vector.tensor_copy(v2v[:, h, 0, :], u2v[:, h, 0, :])
                for t in range(1, T):
                    nc.vector._custom_dve(
                        LIF_STEP,
                        out=v2v[:, h, t, :],
                        in0=v2v[:, h, t - 1, :],
                        in1=u2v[:, h, t, :],
                        s0=ALPHA,
                        s1=THR,
                    )
                    if t in t_flush:
                        flush(*t_flush[t])
                flush(90, T)
            else:
                nc.vector.memset(ym, 0.0)
                for t in range(T):
                    nc.vector.scalar_tensor_tensor(
                        out=v2v[:, h, t, :],
                        in0=ym,
                        scalar=-ALPHA,
                        in1=u2v[:, h, t, :],
                        op0=mm,
                        op1=ad,
                    )
                    nc.vector.scalar_tensor_tensor(
                        out=ym,
                        in0=v2v[:, h, t, :],
                        scalar=THR,
                        in1=v2v[:, h, t, :],
                        op0=mybir.AluOpType.is_ge,
                        op1=mybir.AluOpType.subtract,
                    )
                    if t in t_flush:
                        flush(*t_flush[t])
                flush(90, T)

        for rep in range(reps):
            if rep > 0:
                tc.strict_bb_all_engine_barrier()
            for st in range(ST):
                xsup = xpool.tile([128, K4 * F * T], _DT)
                xsv = xsup.rearrange("p (k4 ct) -> p k4 ct", k4=K4)
                if st < ST - 1:
                    nc.sync.dma_start(out=xsv, in_=xs[st])
                    weighted_and_scan(st, xsv, [0, 1, 2, 3])
                else:
                    # last supertile: k4-granular DMA + compute for a short
                    # tail; the final k4 is further split into 8-feature DMA
                    # chunks so PE/DVE start before the last byte lands.
                    for k4 in range(K4 - 1):
                        nc.sync.dma_start(
                            out=xsv[:, k4, :], in_=xs[st][:, k4, :]
                        )
                        weighted_and_scan(st, xsv, [k4])
                    k4 = K4 - 1
                    for fc in range(4):
                        fs = slice(fc * 8 * T, (fc + 1) * 8 * T)
                        nc.sync.dma_start(
                            out=xsv[:, k4, fs], in_=xs[st][:, k4, fs]
                        )
                    weighted_and_scan(st, xsv, [k4])
                if st == ST // 2 - 1:
                    lif_half(0)
            lif_half(1)

    nc.finalize()
    return nc


class _Launcher:
    """Compiled SPMD launcher (mirrors bass2jax.run_bass_via_pjrt but keeps
    the jitted executable so repeat calls don't recompile)."""

    def __init__(self, nc: bass.Bass, donate: bool = True):
        from jax.experimental.shard_map import shard_map
        from jax.sharding import Mesh, PartitionSpec
        from concourse.bass2jax import (
            _bass_exec_p,
            install_neuronx_cc_hook,
            partition_id_tensor,
        )

        install_neuronx_cc_hook()
        self.nc = nc
        partition_name = (
            nc.partition_id_tensor.name if nc.partition_id_tensor else None
        )
        in_names: list[str] = []
        out_names: list[str] = []
        out_avals: list[jax.core.ShapedArray] = []
        zero_shapes: list[tuple] = []
        for alloc in nc.m.functions[0].allocations:
            if not isinstance(alloc, mybir.MemoryLocationSet):
                continue
            name = alloc.memorylocations[0].name
            if alloc.kind == "ExternalInput":
                if name != partition_name:
                    in_names.append(name)
            elif alloc.kind == "ExternalOutput":
                out_names.append(name)
                shape = tuple(alloc.tensor_shape)
                dtype = mybir.dt.np(alloc.dtype)
                out_avals.append(jax.core.ShapedArray(shape, dtype))
                zero_shapes.append((shape, dtype))
        self.in_names = list(in_names)
        self.out_names = out_names
        self.out_avals = out_avals
        self.zero_shapes = zero_shapes
        n_params = len(in_names)
        all_in_names = list(in_names) + list(out_names)
        if partition_name is not None:
            all_in_names.append(partition_name)

        def _body(*args):
            operands = list(args)
            if partition_name is not None:
                operands.append(partition_id_tensor())
            outs = _bass_exec_p.bind(
                *operands,
                out_avals=tuple(out_avals),
                in_names=tuple(all_in_names),
                out_names=tuple(out_names),
                lowering_input_output_aliases=(),
                sim_require_finite=True,
                sim_require_nnan=True,
                nc=nc,
            )
            return tuple(outs)

        devices = jax.devices()[:N_CORES]
        self.mesh = Mesh(np.asarray(devices), ("core",))
        n_outs = len(out_names)
        donate_argnums = (
            tuple(range(n_params, n_params + n_outs)) if donate else ()
        )
        in_specs = (PartitionSpec("core"),) * (n_params + n_outs)
        out_specs = (PartitionSpec("core"),) * n_outs
        self.sharded = jax.jit(
            shard_map(
                _body,
                mesh=self.mesh,
                in_specs=in_specs,
                out_specs=out_specs,
                check_rep=False,
            ),
            donate_argnums=donate_argnums,
            keep_unused=True,
        )

    def zeros(self):
        return [
            np.zeros((N_CORES * s[0], *s[1:]), d) for (s, d) in self.zero_shapes
        ]

    def __call__(self, concat_inputs):
        out_arrs = self.sharded(*concat_inputs, *self.zeros())
        return [np.asarray(o) for o in out_arrs]


_launchers: dict[tuple, _Launcher] = {}


def _get_launcher(reps: int = 1, donate: bool = True) -> _Launcher:
    key = (reps, donate)
    if key not in _launchers:
        _launchers[key] = _Launcher(_build_program(reps), donate=donate)
    return _launchers[key]


def _make_wsel(w: np.ndarray) -> np.ndarray:
    ws = np.zeros((F_PE, 128, 128), dtype=np.float32)
    idx = np.arange(128)
    for c in range(F_PE):
        ws[c, idx, idx] = w[0, c]
    return ws


def _unscramble(full_out: np.ndarray) -> np.ndarray:
    # per-core col = h*1600 + t*16 + kh;  batch-within-core = (h*16+kh)*128 + p
    return (
        full_out.reshape(N_CORES, 128, 2, T, HALF_KB)
        .transpose(0, 2, 4, 1, 3)
        .reshape(B_FULL, 1, T)
    )


def _prep_inputs(x, w):
    x = np.ascontiguousarray(np.asarray(x, dtype=np.float32))
    w = np.ascontiguousarray(np.asarray(w, dtype=np.float32))
    assert x.shape == (B_FULL, 2, 4, 4, T), x.shape
    assert w.shape == (1, F), w.shape
    wsc = (np.float32(ONE_MINUS_ALPHA) * w).astype(np.float32)
    ws = _make_wsel(wsc)
    ws_rep = np.broadcast_to(ws, (N_CORES, *ws.shape)).reshape(
        N_CORES * F_PE, 128, 128
    )
    wb = np.broadcast_to(wsc[0], (128, F))
    wb_rep = np.broadcast_to(wb, (N_CORES, 128, F)).reshape(N_CORES * 128, F)
    return [
        x,
        np.ascontiguousarray(ws_rep),
        np.ascontiguousarray(wb_rep),
    ]


def run(x, w, reps: int = 1):
    launcher = _get_launcher(reps)
    concat_in = _prep_inputs(x, w)
    # input order must match the BIR ExternalInput declaration order
    assert launcher.in_names == ["x", "wsel", "wb"], launcher.in_names
    outs = launcher(concat_in)
    return _unscramble(outs[0])


def kernel(x, w):
    return run(x, w, reps=1)
